# revision 41
# baseline (speedup 1.0000x reference)
"""CascadeCornerPooling TRN2 kernel (fp8 DoubleRow conv1/conv2/conv3).

Data-parallel over batch: 16 images across 8 NeuronCores (2 per core).
Per image (NCHW, C_in=256, C_out=128, H=W=128):
    up    = relu(bn1(conv3x3(x, w_up)))
    up    = reverse-cummax over H          (TopPool)
    down  = relu(bn2(conv3x3(x, w_down)))
    merge = bn3(conv3x3(up + down, w_p))
    out   = reverse-cummax over W          (LeftPool)

Implementation: H-bands of 16 rows processed bottom-up.
 - conv1/conv2: x and weights quantized to fp8-e4m3 on host (weights
   pre-scaled by 64, folded back via the BN scale). Each 3x3 tap is ONE
   DoubleRow matmul with the two Cin-128-chunks as the two K-subtiles
   (K=256 per instruction, 0.5 cycles/row). The whole padded image is
   host-prepacked as [slot 0..129][chunk 0..1][144] fp8 and DMA'd in
   bottom-up pieces.
 - conv3 ALSO runs in fp8 DoubleRow with two tricks that keep it inside
   the accuracy gate:
     (a) per-band per-channel mean subtraction: mu = e4m3-rounded mean of
         the band's merge tile; the fp8 input is e4m3(mg - mu), which has
         ~3x smaller absolute quantization error than e4m3(mg).  The
         correction term conv3(mu-field) is a per-band per-channel
         CONSTANT (pad cols/rows of the fp8 tile hold e4m3(-mu) == -mu
         exactly, making every output see all 9 taps of mu), computed by
         a tiny K=128 ap=1 matmul (wsum9 @ mu) and folded into the
         existing PSUM-evacuation bias.
     (b) two-term fp8 weights: w_p ~ whi + wlo (both e4m3 at the same x64
         scale, wlo the residual) -> 18 tap-passes packed as 9 DoubleRow
         matmuls (chunk pairs along kh share a 144-aligned row stride).
   Net conv3 PE cost: 4.5N vs bf16's 9N -> ~60us saved per core.
 - ub/dn/merge intermediates are fp16 (same DVE cost as bf16, finer
   rounding).  BN+ReLU fused into ScalarE PSUM evacuation.
 - The fp16->fp8 mu-shifted conversion runs on the idle GpSimd engine in
   row-pieces, one band behind the merge; the merge is split 8+8 with mu
   sampled from the first half so the conversion starts early and the
   next band's conv3 never waits.
 - TopPool: in-place log-shift suffix-max over 17 rows.
 - LeftPool: the BN3 evacuation writes rows in natural order; ONE masked
   tensor_tensor_scan per band runs RIGHT-TO-LEFT over reversed APs with
   an additive reset mask (state = max(state + mask, x), mask = -3e38 at
   segment starts), so no positivity offset and no un-reverse pass are
   needed; the y DMA reads the scan result directly.
 - Tail: image 0's top-band conv3 (dedicated mg8T/scT tiles) is deferred
   and interleaved with image 1's top-band conv3 in shrinking chunks so
   the PE keeps streaming through the drain.
"""

import numpy as np
import ml_dtypes

import concourse.bass as bass
import concourse.tile as tile
from concourse import mybir
from concourse.bass_utils import run_bass_kernel_spmd

F32 = mybir.dt.float32
F16 = mybir.dt.float16
F8 = mybir.dt.float8e4
HDT = F16                      # half dtype for ub/dn/merge intermediates
E4M3 = ml_dtypes.float8_e4m3   # TRN FP8_EXP4-compatible (max +-240)
DR = mybir.MatmulPerfMode.DoubleRow
AX = mybir.AxisListType
ALU = mybir.AluOpType
ACT = mybir.ActivationFunctionType

N_CORES = 8
IMG_PER_CORE = 2
CIN, COUT = 256, 128
H = W = 128
P = 128          # partitions
R = 16           # band rows
NB = H // R      # bands per image
XROW = 144       # fp8 row-chunk stride: [pad, c0..c127, 15 pad], multiple of 16
XSEG = 130 * 2 * XROW  # whole padded image: 130 slots x 2 chunks x 144
MROW = 144       # conv3 fp8 slot stride: [pad(-mu), c0..c127, pad(-mu), 14 junk]
MSEG = 18 * MROW
NEG = -3.0e38    # additive scan-reset value at segment boundaries
WSCALE = 64.0    # weight pre-scale before fp8 quantization (power of 2)
EPS = 1e-5

# conv3 DoubleRow tap pairing: per kw, 3 matmuls with (khA, dk); chunk0 =
# slot q+khA, chunk1 = slot q+khA+dk (stride dk*144, 16-aligned).
C3_MMS = [(0, 1), (0, 2), (1, 1)]
# weight sources per matmul i: (chunk0 hi?, kh0), (chunk1 hi?, kh1)
C3_WSRC = [((True, 0), (True, 1)), ((False, 0), (True, 2)), ((False, 1), (False, 2))]


def _split_waits(nc, max_waits=1):
    """This container's walrus rejects >1 sync-wait per instruction; hoist
    excess waits onto same-engine NOPs inserted just before."""
    for f in nc.m.functions:
        for b in f.blocks:
            new_insts = []
            for inst in b.instructions:
                si = inst.sync_info
                if si is not None and si.on_wait and len(si.on_wait) > max_waits:
                    waits = list(si.on_wait)
                    head, tail_w = waits[:-max_waits], waits[-max_waits:]
                    for ci in range(0, len(head), max_waits):
                        new_insts.append(
                            mybir.InstNoOp(
                                name=f"{inst.name}-wsplit{ci}",
                                engine=inst.engine,
                                bass_nofuse=True,
                                sync_info=mybir.SyncInfo(
                                    on_wait=head[ci : ci + max_waits], on_update=[]
                                ),
                            )
                        )
                    inst.sync_info = mybir.SyncInfo(
                        on_wait=tail_w, on_update=list(si.on_update)
                    )
                new_insts.append(inst)
            b.instructions[:] = new_insts


def build_nc(nrep=1, no_pool=False, no_scan=False, pool_engine="vector"):
    nc = bass.Bass("TRN2", target_bir_lowering=False, debug=False)

    x_d = nc.dram_tensor("x", [IMG_PER_CORE, P, XSEG], F8, kind="ExternalInput").ap()
    wu_d = nc.dram_tensor("wu", [P, 2 * 9 * P], F8, kind="ExternalInput").ap()
    wd_d = nc.dram_tensor("wd", [P, 2 * 9 * P], F8, kind="ExternalInput").ap()
    wp8_d = nc.dram_tensor("wp8", [P, 9 * 2 * P], F8, kind="ExternalInput").ap()
    wp0_d = nc.dram_tensor("wp0", [P, 6 * 2 * P], F8, kind="ExternalInput").ap()
    ws9_d = nc.dram_tensor("ws9", [P, P], F32, kind="ExternalInput").ap()
    wsb_d = nc.dram_tensor("wsb", [P, P], F32, kind="ExternalInput").ap()
    bn_d = nc.dram_tensor("bn", [P, 7], F32, kind="ExternalInput").ap()
    y_d = nc.dram_tensor("y", [IMG_PER_CORE, COUT, H, W], F32, kind="ExternalOutput").ap()

    with tile.TileContext(nc) as tc:
        with (
            tc.tile_pool(name="const", bufs=1) as cp,
            tc.tile_pool(name="band", bufs=1) as bp,
            tc.tile_pool(name="ps", bufs=3, space="PSUM") as ps,
        ):
            # ---- constants ----
            wu_t = cp.tile([P, 2 * 9 * P], F8)
            wd_t = cp.tile([P, 2 * 9 * P], F8)
            wp8_t = cp.tile([P, 9 * 2 * P], F8)
            wp0_t = cp.tile([P, 6 * 2 * P], F8)
            ws9_t = cp.tile([P, P], F32)
            wsb_t = cp.tile([P, P], F32)
            wu_v = wu_t.rearrange("k (s c m) -> k s c m", s=9, c=2, m=P)
            wd_v = wd_t.rearrange("k (s c m) -> k s c m", s=9, c=2, m=P)
            wp8_v = wp8_t.rearrange("k (s c m) -> k s c m", s=9, c=2, m=P)
            wp0_v = wp0_t.rearrange("k (s c m) -> k s c m", s=6, c=2, m=P)

            bn_t = cp.tile([P, 7], F32)
            s1, b1 = bn_t[:, 0:1], bn_t[:, 1:2]
            s2, b2 = bn_t[:, 2:3], bn_t[:, 3:4]
            s3d, b3a = bn_t[:, 4:5], bn_t[:, 5:6]   # s3/64, b3
            s3t = bn_t[:, 6:7]                       # true s3 (bias matmul fold)

            zf = cp.tile([P, 256], HDT)              # f16 zeros (pad/halo fills)
            nc.vector.memset(zf[:], 0.0)
            zf36 = zf[:, 0:36].rearrange("p (a c) -> p a c", a=18, c=2)
            zf256 = zf[:, 0:256].rearrange("p (a c) -> p a c", a=2, c=128)

            # additive scan-reset mask: NEG at the first element of each
            # W-segment in scan order, 0 elsewhere
            mask = cp.tile([P, R * W], F32)
            nc.vector.memset(mask[:], 0.0)
            nc.vector.memset(mask[:, 0::W], NEG)

            # ---- PE warmup: dummy matmuls on a never-written tile bridge
            # the initial x-DMA wait so the p-state ramp (3us to full clock)
            # completes before the first real conv matmul; the garbage psum
            # is never read ----
            junk = cp.tile([P, 256], HDT)
            nc.gpsimd.memset(junk[:], 0.0)
            for _ in range(17):
                warm = ps.tile([P, 4 * W], F32, name="warm", tag="pd")
                nc.tensor.matmul(
                    warm[:, 0:256], junk[:, 0:128], junk[:, 0:256],
                    start=True, stop=True,
                )

            # ---- band tiles (manual ping-pong) ----
            xi = [bp.tile([P, XSEG], F8, name=f"xi{j}", tag=f"xi{j}") for j in range(IMG_PER_CORE)]
            ub = [bp.tile([P, 17 * W], HDT, name=f"ub{j}", tag=f"ub{j}") for j in range(2)]
            dn = [bp.tile([P, R * W], HDT, name=f"dn{j}", tag=f"dn{j}") for j in range(2)]
            mgf = [bp.tile([P, R * W], HDT, name=f"mgf{j}", tag=f"mgf{j}") for j in range(2)]
            mg8 = [bp.tile([P, MSEG], F8, name=f"mg8{j}", tag=f"mg8{j}") for j in range(2)]
            mg8T = bp.tile([P, MSEG], F8, name="mg8T", tag="mg8T")
            sc = [bp.tile([P, R * W], F32, name=f"sc{j}", tag=f"sc{j}") for j in range(2)]
            sc0 = bp.tile([P, W], F32)
            scT = bp.tile([P, R * W], F32, name="scT", tag="scT")

            # per-band mu chain tiles (ping-pong) + deferred-band biases
            musA = [bp.tile([P, 1], F32, name=f"musA{j}") for j in range(2)]
            musum = [bp.tile([P, 1], F32, name=f"musum{j}") for j in range(2)]
            mu8q = [bp.tile([P, 1], F8, name=f"mu8q{j}") for j in range(2)]
            muq = [bp.tile([P, 1], F32, name=f"muq{j}") for j in range(2)]
            negmu = [bp.tile([P, 1], F32, name=f"negmu{j}") for j in range(2)]
            biasb = [bp.tile([P, 1], F32, name=f"biasb{j}") for j in range(2)]
            biasT = bp.tile([P, 1], F32, name="biasT")
            bias0 = bp.tile([P, 1], F32, name="bias0")

            def mgfv(j):
                return mgf[j].rearrange("p (r c) -> p r c", r=R, c=W)

            def m8v(t_):
                return t_.rearrange("p (a c) -> p a c", a=18, c=MROW)

            def rhs_dr(n, row0, nrows, kw):
                """conv1/2 DoubleRow rhs: [p, 2(chunk), nrows, W] fp8 view."""
                base = xi[n].rearrange("p (r k c) -> p k r c", r=130, k=2, c=XROW)
                return base[:, :, row0 : row0 + nrows, kw : kw + W]

            def rhs_c3(t_, q, nr, khA, dk, kw):
                """conv3 DoubleRow rhs: [p, 2(chunk @ dk*144), nr(@144), W]
                overlapping strided view of the fp8 mu-shifted tile."""
                v = m8v(t_)
                s = v[:, q + khA : q + khA + dk + 1 : dk, kw : kw + W]
                u = s.unsqueeze(2).copy()
                u.ap[2] = [MROW, nr]
                return u

            def conv12_mms(psum, w_v, n, row0, nrows):
                """9 DoubleRow matmuls (one per tap, K=256 via 2 chunks)."""
                for kh in range(3):
                    for kw in range(3):
                        s = kh * 3 + kw
                        nc.tensor.matmul(
                            psum[:, : nrows * W],
                            w_v[:, s, :, :],
                            rhs_dr(n, row0 + kh, nrows, kw),
                            start=(s == 0),
                            stop=(s == 8),
                            perf_mode=DR,
                        )

            def conv3_mms(psum, t_, q, nr):
                """9 DoubleRow matmuls: 18 (tap x hi/lo) passes, kh-paired."""
                s = 0
                for kw in range(3):
                    for khA, dk in C3_MMS:
                        nc.tensor.matmul(
                            psum[:, : nr * W],
                            wp8_v[:, s, :, :],
                            rhs_c3(t_, q, nr, khA, dk, kw),
                            start=(s == 0),
                            stop=(s == 8),
                            perf_mode=DR,
                        )
                        s += 1

            def rscan(sc_ap, ne):
                """segmented suffix-max: right-to-left scan over reversed APs
                with the additive reset mask."""
                if no_scan:
                    return
                rv = sc_ap[:, ::-1]
                nc.vector.tensor_tensor_scan(
                    rv, mask[:, :ne], rv, 0.0, op0=ALU.add, op1=ALU.max,
                )

            def mu_chain(j, dn_rows):
                """band mu = mean(pooled ub) + mean(dn) over subsampled rows
                (computable BEFORE the merge), e4m3-rounded, then +-mu f32."""
                nsamp = dn_rows.shape[1] * W
                nc.vector.tensor_reduce(musum[j][:], dn_rows, AX.XY, ALU.add)
                nc.vector.tensor_add(musum[j][:], musum[j][:], musA[j][:])
                nc.vector.tensor_scalar_mul(mu8q[j][:], musum[j][:], 1.0 / nsamp)
                nc.vector.tensor_copy(muq[j][:], mu8q[j][:])
                nc.vector.tensor_scalar_mul(negmu[j][:], muq[j][:], -1.0)

            def bias_chain(mu_ap, w_t, out_ap, tag="pc"):
                """bias = s3 * (w_sums @ mu) + b3   (tiny ap=1 fp32 matmul;
                evacuated on ScalarE to keep the DVE queue clear)."""
                pb = ps.tile([P, 4 * W], F32, name="pb", tag=tag,
                             bufs=2 if tag == "pc" else None)
                nc.tensor.matmul(pb[:, 0:1], w_t[:], mu_ap, start=True, stop=True)
                nc.scalar.activation(
                    out_ap, pb[:, 0:1], ACT.Identity, bias=b3a, scale=s3t
                )

            def convert_pads(t_, neg_ap):
                nc.gpsimd.tensor_scalar_add(
                    m8v(t_)[:, :, 0 : W + 2 : W + 1], zf36, neg_ap
                )

            def convert_rows(t_, neg_ap, src_j, r0, r1):
                nc.gpsimd.tensor_scalar_add(
                    m8v(t_)[:, r0:r1, 1 : 1 + W], mgfv(src_j)[:, r0:r1, :], neg_ap
                )

            def convert_halo(t_, neg_ap, src_j):
                """halo slots 16,17 = rows 0,1 of the band below (single quant
                from its fp16 tile), or e4m3(-mu) outside the image."""
                if src_j is None:
                    nc.gpsimd.tensor_scalar_add(
                        m8v(t_)[:, 16:18, 1 : 1 + W], zf256, neg_ap
                    )
                else:
                    nc.gpsimd.tensor_scalar_add(
                        m8v(t_)[:, 16:18, 1 : 1 + W], mgfv(src_j)[:, 0:2, :], neg_ap
                    )

            rep_ctx = tc.For_i(0, nrep, 1) if nrep > 1 else None
            if rep_ctx is not None:
                rep_ctx.__enter__()

            def conv3_group(n, k, t_, bias_ap, sc_t, q, nr, tag, bufs,
                            dma_span=None, defer_finish=False):
                """One conv3 group with its own LeftPool chain (tail style)."""
                h0 = H - (k + 1) * R
                scv = sc_t.rearrange("p (r c) -> p r c", r=R, c=W)
                pc = ps.tile([P, 4 * W], F32, name=tag, tag=tag, bufs=bufs)
                conv3_mms(pc, t_, q, nr)
                nc.scalar.activation(
                    scv[:, q : q + nr, :], pc[:, : nr * W],
                    ACT.Identity, bias=bias_ap, scale=s3d,
                )

                def finish():
                    rscan(sc_t[:, q * W : (q + nr) * W], nr * W)
                    if dma_span is not None:
                        q0, nd = dma_span
                        nc.sync.dma_start(
                            y_d[n, :, h0 + 1 + q0 : h0 + 1 + q0 + nd, :],
                            scv[:, q0 : q0 + nd, :],
                        )
                if defer_finish:
                    return finish
                finish()

            def emit_piece(k, q, dve=False):
                """late fp8-conversion pieces of band k, emitted just before
                the conv3 group that first reads them (tile-granular waits
                would otherwise stall every group on the LAST convert op).
                dve=True runs the piece on the vector engine instead of
                GpSimd (drain: both engines convert in parallel)."""
                j = k % 2
                eng = nc.vector if dve else nc.gpsimd
                def rows(r0, r1):
                    eng.tensor_scalar_add(
                        m8v(mg8[j])[:, r0:r1, 1 : 1 + W],
                        mgfv(j)[:, r0:r1, :], negmu[j][:],
                    )
                if q == 4:
                    rows(6, 10)
                elif q == 8:
                    rows(10, 14)
                elif q == 12:
                    rows(14, 16)
                    convert_halo(mg8[j], negmu[j][:], None if k == 0 else 1 - j)

            def conv3_band(n, k, defer_finish=False, pieces_done=False):
                """conv3 + LeftPool + output DMA for band k (lagged one band)."""
                h0 = H - (k + 1) * R
                j = k % 2
                n_out = R if k > 0 else R - 1
                scv = sc[j].rearrange("p (r c) -> p r c", r=R, c=W)
                q = 0
                while q < n_out:
                    if not pieces_done:
                        emit_piece(k, q)
                    nr = min(4, n_out - q)
                    pc = ps.tile([P, 4 * W], F32, name="pc", tag="pc", bufs=2)
                    conv3_mms(pc, mg8[j], q, nr)
                    nc.scalar.activation(
                        scv[:, q : q + nr, :], pc[:, : nr * W],
                        ACT.Identity, bias=biasb[j], scale=s3d,
                    )
                    q += nr
                if defer_finish:
                    return

                def finish():
                    rscan(sc[j][:, : n_out * W], n_out * W)
                    nc.sync.dma_start(
                        y_d[n, :, h0 + 1 : h0 + 1 + n_out, :], scv[:, 0:n_out, :]
                    )
                finish()

            def conv3_band_finish(n, k):
                h0 = H - (k + 1) * R
                j = k % 2
                n_out = R if k > 0 else R - 1
                scv = sc[j].rearrange("p (r c) -> p r c", r=R, c=W)
                # chunked scans: the greedy scheduler can interleave the mu /
                # merge chain between them instead of stalling 2.2us
                for q in range(0, n_out, 4):
                    nr = min(4, n_out - q)
                    rscan(sc[j][:, q * W : (q + nr) * W], nr * W)
                nc.sync.dma_start(
                    y_d[n, :, h0 + 1 : h0 + 1 + n_out, :], scv[:, 0:n_out, :]
                )

            def p0_block(n, t_):
                """out row 0: taps kh=1,2 on slots 0,1 (6 DR matmuls)."""
                p0 = ps.tile([P, 4 * W], F32, name="p0", tag="pc", bufs=2)
                for s in range(6):
                    kw = s // 2
                    nc.tensor.matmul(
                        p0[:, :W], wp0_v[:, s, :, :], rhs_c3(t_, -1, 1, 1, 1, kw),
                        start=(s == 0), stop=(s == 5), perf_mode=DR,
                    )
                nc.scalar.activation(
                    sc0[:], p0[:, :W], ACT.Identity, bias=bias0, scale=s3d,
                )
                rscan(sc0, W)
                nc.sync.dma_start(
                    y_d[n, :, 0:1, :], sc0[:].rearrange("p (r c) -> p r c", r=1, c=W)
                )

            # ---- DMA schedule: weights first (no mid-conv stalls), then the
            # first conv group's x slots, then the rest bottom-up ----
            RW = 2 * XROW
            nc.sync.dma_start(wu_t[:], wu_d[:])
            nc.sync.dma_start(xi[0][:, 112 * RW : 118 * RW], x_d[0, :, 112 * RW : 118 * RW])
            nc.sync.dma_start(bn_t[:], bn_d[:])
            nc.sync.dma_start(xi[0][:, 118 * RW : 122 * RW], x_d[0, :, 118 * RW : 122 * RW])
            nc.sync.dma_start(xi[0][:, 122 * RW : XSEG], x_d[0, :, 122 * RW : XSEG])
            nc.sync.dma_start(wd_t[:], wd_d[:])
            nc.sync.dma_start(xi[0][:, 76 * RW : 112 * RW], x_d[0, :, 76 * RW : 112 * RW])
            nc.sync.dma_start(wp8_t[:], wp8_d[:])
            nc.sync.dma_start(wp0_t[:], wp0_d[:])
            nc.sync.dma_start(ws9_t[:], ws9_d[:])
            nc.sync.dma_start(wsb_t[:], wsb_d[:])
            for a_, b_ in ((44 * RW, 76 * RW), (0, 44 * RW)):
                nc.sync.dma_start(xi[0][:, a_:b_], x_d[0, :, a_:b_])
            for n_ in range(1, IMG_PER_CORE):
                for a_, b_ in ((80 * RW, XSEG), (0, 80 * RW)):
                    nc.sync.dma_start(xi[n_][:, a_:b_], x_d[n_, :, a_:b_])

            for n in range(IMG_PER_CORE):
                for k in range(NB):
                    h0 = H - (k + 1) * R
                    j = k % 2

                    # ---- conv1 -> ub rows 0..15 (fp8 DR, BN+ReLU) ----
                    # (emitted FIRST so the PE has work while the previous
                    # band's fp8 conversion finishes on GpSimd)
                    for t in range(R // 4):
                        pu = ps.tile([P, 4 * W], F32, name="pu", tag="pu")
                        conv12_mms(pu, wu_v, n, h0 + 4 * t, 4)
                        nc.scalar.activation(
                            ub[j][:, 4 * t * W : 4 * (t + 1) * W], pu[:],
                            ACT.Relu, bias=b1, scale=s1,
                        )

                    # ---- previous band's conv3 (bias matmul first) ----
                    # (at the final band it moves into the drain tail, after
                    # the mu chain, so the greedy per-engine scheduler cannot
                    # slot its big scan ahead of the mu ops)
                    if k > 0:
                        jp = (k - 1) % 2
                        bias_chain(muq[jp][:], ws9_t, biasb[jp][:])
                        # at the final band, band k-1's scan+DMA are deferred
                        # past the merge so the mu chain heads the DVE queue
                        defer = k == NB - 1 and n == IMG_PER_CORE - 1
                        conv3_band(n, k - 1, defer_finish=defer)
                    elif n > 0:
                        # deferred chains of the previous image's top band:
                        # bias for mg8T, fp8 conversion of mg8T, row-0 bias
                        bias_chain(muq[1][:], ws9_t, biasT[:])
                        bias_chain(muq[1][:], wsb_t, bias0[:])
                        convert_pads(mg8T, negmu[1][:])
                        convert_rows(mg8T, negmu[1][:], 1, 0, 16)
                        convert_halo(mg8T, negmu[1][:], 0)

                    # ---- carry row (slot 16) ----
                    if k == 0:
                        nc.vector.memset(ub[j][:, 16 * W :], 0.0)
                    else:
                        nc.vector.tensor_copy(ub[j][:, 16 * W :], ub[1 - j][:, 0:W])

                    # ---- TopPool: in-place suffix max over 17 rows ----
                    if not no_pool:
                        peng = getattr(nc, pool_engine)
                        for s in (1, 2, 4, 8, 16):
                            nrows = 17 - s
                            peng.tensor_max(
                                ub[j][:, : nrows * W],
                                ub[j][:, : nrows * W],
                                ub[j][:, s * W : 17 * W],
                            )
                    # partial mu: subsampled sum of the pooled ub rows
                    ubv = ub[j].rearrange("p (r c) -> p r c", r=17, c=W)
                    nc.vector.tensor_reduce(
                        musA[j][:], ubv[:, 2:R:4, :], AX.XY, ALU.add
                    )

                    # ---- conv2 -> dn (fp8 DR, BN+ReLU) ----
                    for t in range(R // 4):
                        pd = ps.tile([P, 4 * W], F32, name="pd", tag="pd")
                        conv12_mms(pd, wd_v, n, h0 + 4 * t, 4)
                        nc.scalar.activation(
                            dn[j][:, 4 * t * W : 4 * (t + 1) * W], pd[:],
                            ACT.Relu, bias=b2, scale=s2,
                        )

                    # ---- previous image's row 0 (needs mg8T conversion) ----
                    if k == 0 and n > 0:
                        p0_block(n - 1, mg8T)

                    # ---- mu (full-band sample, ready before the merge), then
                    # merge = pooled + down (split 8+8) and piecewise
                    # fp16 -> fp8 mu-shifted conversion ----
                    mv = mgfv(j)
                    dnv = dn[j].rearrange("p (r c) -> p r c", r=R, c=W)
                    with tc.high_priority(offset=600):
                        mu_chain(j, dnv[:, 2:R:4, :])
                        nc.vector.tensor_add(mv[:, 0:6, :], ubv[:, 0:6, :], dnv[:, 0:6, :])
                    nc.vector.tensor_add(mv[:, 6:R, :], ubv[:, 6:R, :], dnv[:, 6:R, :])

                    last_band = k == NB - 1
                    if not last_band or n == IMG_PER_CORE - 1:
                        # early pieces only; B/C/D + halo are emitted just
                        # before the conv3 groups that read them
                        with tc.high_priority(offset=600):
                            convert_pads(mg8[j], negmu[j][:])
                            convert_rows(mg8[j], negmu[j][:], j, 0, 6)
                    # (top band of a non-final image: conversion deferred to
                    # the next image's first band, into mg8T)

                # ---- drain tail ----
                if n == IMG_PER_CORE - 1:
                    jl = (NB - 1) % 2
                    # band NB-2's deferred scan + y DMA (now that the mu
                    # chain had the DVE first)
                    conv3_band_finish(n, NB - 2)
                    # PE chews the long-ready mg8T groups while the final
                    # band's mu/convert chain completes on DVE/GpSimd; their
                    # scans + y DMAs are deferred out of the mu-critical
                    # window (they would clutter the DVE/semaphore space)
                    conv3_group(0, NB - 1, mg8T, biasT, scT, 0, 4, "pu", None, (0, 4))
                    conv3_group(0, NB - 1, mg8T, biasT, scT, 4, 4, "pu", None, (4, 4))
                    conv3_group(0, NB - 1, mg8T, biasT, scT, 8, 2, "pu", None, None)
                    conv3_group(0, NB - 1, mg8T, biasT, scT, 10, 2, "pu", None, (8, 4))
                    conv3_group(0, NB - 1, mg8T, biasT, scT, 12, 2, "pu", None, (12, 2))
                    conv3_group(0, NB - 1, mg8T, biasT, scT, 14, 2, "pu", None, (14, 2))
                    bias_chain(muq[jl][:], ws9_t, biasb[jl][:], tag="pd")
                    bias_chain(muq[jl][:], wsb_t, bias0[:], tag="pd")
                    conv3_group(n, NB - 1, mg8[jl], biasb[jl], sc[jl],
                                0, 4, "pc", 2, (0, 4))
                    p0_block(n, mg8[jl])
                    emit_piece(NB - 1, 4, dve=True)
                    conv3_group(n, NB - 1, mg8[jl], biasb[jl], sc[jl],
                                4, 4, "pc", 2, (4, 4))
                    emit_piece(NB - 1, 8)
                    conv3_group(n, NB - 1, mg8[jl], biasb[jl], sc[jl],
                                8, 2, "pc", 2, None)
                    emit_piece(NB - 1, 12)
                    conv3_group(n, NB - 1, mg8[jl], biasb[jl], sc[jl],
                                10, 2, "pc", 2, (8, 4))
                    conv3_group(n, NB - 1, mg8[jl], biasb[jl], sc[jl],
                                12, 2, "pc", 2, None)
                    conv3_group(n, NB - 1, mg8[jl], biasb[jl], sc[jl],
                                14, 2, "pc", 2, (12, 4))
            if rep_ctx is not None:
                rep_ctx.__exit__(None, None, None)

    _split_waits(nc, max_waits=1)
    return nc


_CACHE = {}


def _get_nc():
    if "nc" not in _CACHE:
        _CACHE["nc"] = build_nc()
    return _CACHE["nc"]


def _host_prep(w_up, up_gamma, up_beta, up_mean, up_var,
               w_down, down_gamma, down_beta, down_mean, down_var,
               w_p, p_gamma, p_beta, p_mean, p_var):
    def fold(gamma, beta, mean, var):
        inv = gamma / np.sqrt(var + EPS)
        return inv.astype(np.float32), (beta - mean * inv).astype(np.float32)

    s1, b1 = fold(up_gamma, up_beta, up_mean, up_var)
    s2, b2 = fold(down_gamma, down_beta, down_mean, down_var)
    s3, b3 = fold(p_gamma, p_beta, p_mean, p_var)
    # conv weights are stored as fp8(WSCALE*w); undo via the BN scale
    bn = np.stack([s1 / WSCALE, b1, s2 / WSCALE, b2, s3 / WSCALE, b3, s3],
                  axis=1).astype(np.float32)

    def prep_w2(w):  # (COUT, CIN, 3, 3) -> fp8 [cin128, (s, chunk, cout128)]
        a = w.transpose(1, 2, 3, 0).reshape(2, P, 3, 3, COUT)   # (chunk,k,kh,kw,m)
        a = a.transpose(1, 2, 3, 0, 4)                          # (k,kh,kw,chunk,m)
        a = np.ascontiguousarray(a.reshape(P, 2 * 9 * COUT)).astype(np.float32)
        return (a * WSCALE).astype(E4M3)

    # conv3: two-term fp8 weights, kh-paired DoubleRow packing
    whi = (WSCALE * w_p).astype(E4M3)                            # (m,k,kh,kw)
    wlo = (WSCALE * w_p - whi.astype(np.float32)).astype(E4M3)
    hi_t = whi.astype(np.float32).transpose(1, 2, 3, 0)          # (k,kh,kw,m)
    lo_t = wlo.astype(np.float32).transpose(1, 2, 3, 0)
    wp8 = np.zeros((P, 9, 2, COUT), np.float32)
    for kw in range(3):
        for i, ((hi0, kh0), (hi1, kh1)) in enumerate(C3_WSRC):
            s = kw * 3 + i
            wp8[:, s, 0, :] = (hi_t if hi0 else lo_t)[:, kh0, kw, :]
            wp8[:, s, 1, :] = (hi_t if hi1 else lo_t)[:, kh1, kw, :]
    wp8 = np.ascontiguousarray(wp8.reshape(P, 9 * 2 * COUT)).astype(E4M3)
    wp0 = np.zeros((P, 6, 2, COUT), np.float32)
    for kw in range(3):
        wp0[:, kw * 2 + 0, 0, :] = hi_t[:, 1, kw, :]
        wp0[:, kw * 2 + 0, 1, :] = hi_t[:, 2, kw, :]
        wp0[:, kw * 2 + 1, 0, :] = lo_t[:, 1, kw, :]
        wp0[:, kw * 2 + 1, 1, :] = lo_t[:, 2, kw, :]
    wp0 = np.ascontiguousarray(wp0.reshape(P, 6 * 2 * COUT)).astype(E4M3)

    # bias-matmul weight sums (true scale, from the 2-term quantized weights)
    w2t = (whi.astype(np.float32) + wlo.astype(np.float32)) / WSCALE  # (m,k,kh,kw)
    ws9 = np.ascontiguousarray(w2t.sum(axis=(2, 3)).T).astype(np.float32)      # (k,m)
    wsb = np.ascontiguousarray(w2t[:, :, 1:3, :].sum(axis=(2, 3)).T).astype(np.float32)

    return prep_w2(w_up), prep_w2(w_down), wp8, wp0, ws9, wsb, bn


def _prep_x(x8_core):
    """[IMG, 256, H, W] fp8 -> [IMG, P, XSEG] padded SBUF image layout."""
    out = np.zeros((IMG_PER_CORE, P, 130, 2, XROW), E4M3)
    v = x8_core.reshape(IMG_PER_CORE, 2, P, H, W)
    out[:, :, 1 : 1 + H, 0, 1 : 1 + W] = v[:, 0]
    out[:, :, 1 : 1 + H, 1, 1 : 1 + W] = v[:, 1]
    return out.reshape(IMG_PER_CORE, P, XSEG)


def kernel(x, w_up, up_gamma, up_beta, up_mean, up_var,
           w_down, down_gamma, down_beta, down_mean, down_var,
           w_p, p_gamma, p_beta, p_mean, p_var):
    x8 = np.asarray(x, dtype=np.float32).astype(E4M3)
    args = [np.asarray(a, dtype=np.float32) for a in (
        w_up, up_gamma, up_beta, up_mean, up_var,
        w_down, down_gamma, down_beta, down_mean, down_var,
        w_p, p_gamma, p_beta, p_mean, p_var)]
    wu, wd, wp8, wp0, ws9, wsb, bn = _host_prep(*args)

    nc = _get_nc()
    in_maps = []
    for c in range(N_CORES):
        in_maps.append({
            "x": _prep_x(x8[c * IMG_PER_CORE : (c + 1) * IMG_PER_CORE]),
            "wu": wu, "wd": wd, "wp8": wp8, "wp0": wp0,
            "ws9": ws9, "wsb": wsb, "bn": bn,
        })
    res = run_bass_kernel_spmd(nc, in_maps, core_ids=list(range(N_CORES)), trace=False)
    return np.concatenate([res.results[c]["y"] for c in range(N_CORES)], axis=0)


if __name__ == "__main__":
    nc = build_nc()
    n_inst = sum(len(b.instructions) for f in nc.m.functions for b in f.blocks)
    print(f"built: {n_inst} instructions")
